# revision 30
# baseline (speedup 1.0000x reference)
"""Trainium2 Bass kernel for nn_MultiHeadAttention_53266184405720.

Key structural fact: the reference does a raw ``.reshape(h, -1, d)`` on the
[4096, 512] projection output, so "head" h consumes exactly projection rows
[512h, 512h+512) — i.e. sequence rows [512h, 512h+512).  The whole module is
block-diagonal over 512-row sequence blocks: core h computes output rows
[512h, 512h+512) from input rows [512h, 512h+512) plus the (replicated)
weights.  No cross-core communication is needed.

Within a block, with the permutation r~ = c*512 + s (c = column-block of the
projection, s = row), head-reshaped Q/K/V become column-block stacks of the
projection, softmax is permutation-invariant over keys, and the context
unpermutes back into the output projection's contraction.  The transposed
projection layout [64, 512] per column-block c therefore yields every
attention operand as a zero-cost sub-AP.

Perf design (HW-trace driven; baseline 163us -> ~140us):
 - exp offload: ACT (1 elem/cyc/lane) alone needs ~135us for the 16.8M
   score exps and paced the v1 kernel.  Exp is split between ACT (exact
   spline exp) and DVE (one fused tensor_scalar: i16 = round(A*s + B),
   bit-cast to bf16 — Schraudolph in the bf16 exponent/mantissa fields,
   rms err ~1.8% on the offloaded ~47% of keys; HW rounds to nearest).
   Scores issue as packed pairs (tile_position rows 0/64), one pair per
   [128,1024] PSUM tile (3-deep ring, 6 banks), consumers alternating
   ACT/DVE (9:7 / 8:8) so the PE (bf16 scores + AV) becomes the pacer.
 - PE stream: AV matmuls drain in bursts of 4 every other pair — the
   score<->AV transitions expose LDWEIGHTS serialization (~200ns each),
   bursts halve them.  Chunk-0-critical projections (qproj0/kproj0) are
   emitted before the remaining projections; the V chain rides inside
   chunk 0 (scores don't need V until the first AV burst).
 - softmax 1/denom rides the AV matmul as a ones-column (ctx row 64).
   The [1,512] row reciprocal would cost 3.4us/chunk on DVE at 8 cyc/elem;
   instead the row bounces through DRAM as [128,4] (150ns recip) and back
   for the partition-broadcast load (partition-step-0 APs are only legal
   from DRAM here).  Each DMA hop has ~2.8us completion latency, so the
   chain is emitted in three phases across the next two chunks (start:
   copy+2 hops; mid: recip+2 hops; finish: the ctxN multiply) — emitting
   it in one piece head-blocks the strict-FIFO DVE queue and stalls the
   PE ~1-2us per chunk.
 - tail: the output projection accumulates bias (one start=True matmul —
   a later start on the same bank wipes previous slices' has_written) and
   chunks 0..6 into one [128,256] PSUM tile during chunk 7; chunk 7
   projects unnormalized and is scaled per-partition (tensor_scalar with
   an AP scalar, r128b[p,t] = 1/denom[128t+p]) so only 2 DMA hops gate
   the tail instead of 4.
"""

import numpy as np

SEQ = 4096
D = 64
HEADS = 8
B = SEQ // HEADS  # 512 rows per core
N_CORES = 8

# Schraudolph exp for bf16 bit pattern: i16 = round(s * SCH_A + SCH_B),
# bitcast i16 -> bf16 approximates exp(s/8).  SCH_A = (2^7/ln2)/8;
# SCH_B = 127*2^7 - C with C tuned to minimize rms relative error (~1.78%).
SCH_A = 23.083128655
SCH_B = 16256.0 - 7.5

AV_DELAY_P = 2  # pending pairs kept before AV drain (drained 2 at a time)


def _dve_pairs(r1c):
    # even chunks: 7 DVE pairs at odd slots (<=13); odd chunks: 8 at even
    # slots.  Chosen so chunk boundaries keep the two consumers interleaved.
    if r1c % 2 == 0:
        return {1, 3, 5, 7, 9, 11, 13}
    return {0, 2, 4, 6, 8, 10, 12, 14}


_BUILT = None


def _build():
    import concourse.bass as bass
    import concourse.tile as tile
    from concourse import bacc, mybir
    from concourse.masks import make_identity

    f32 = mybir.dt.float32
    f32r = mybir.dt.float32r
    bf16 = mybir.dt.bfloat16
    i16 = mybir.dt.int16
    AF = mybir.ActivationFunctionType
    ALU = mybir.AluOpType

    nc = bacc.Bacc(
        "TRN2",
        target_bir_lowering=False,
        debug=False,
        enable_asserts=True,
        num_devices=N_CORES,
    )

    q = nc.dram_tensor("q", [B, D], f32, kind="ExternalInput").ap()
    k = nc.dram_tensor("k", [B, D], f32, kind="ExternalInput").ap()
    v = nc.dram_tensor("v", [B, D], f32, kind="ExternalInput").ap()
    qw_w = nc.dram_tensor("qw_w", [D, 512], f32, kind="ExternalInput").ap()
    qw_b = nc.dram_tensor("qw_b", [512], f32, kind="ExternalInput").ap()
    kw_w = nc.dram_tensor("kw_w", [D, 512], f32, kind="ExternalInput").ap()
    kw_b = nc.dram_tensor("kw_b", [512], f32, kind="ExternalInput").ap()
    vw_w = nc.dram_tensor("vw_w", [D, 512], f32, kind="ExternalInput").ap()
    vw_b = nc.dram_tensor("vw_b", [512], f32, kind="ExternalInput").ap()
    ow_w = nc.dram_tensor("ow_w", [512, D], f32, kind="ExternalInput").ap()
    ow_b = nc.dram_tensor("ow_b", [D], f32, kind="ExternalInput").ap()
    out = nc.dram_tensor("out", [B, D], f32, kind="ExternalOutput").ap()

    with tile.TileContext(nc) as tc:
        with (
            tc.tile_pool(name="persist", bufs=1) as persist,
            tc.tile_pool(name="inp", bufs=3) as inp,
            tc.tile_pool(name="epool", bufs=6) as epool,
            tc.tile_pool(name="norm", bufs=2) as normp,
            tc.tile_pool(name="outp", bufs=1) as outp,
            tc.tile_pool(name="ps", bufs=3, space="PSUM") as ps,
            tc.tile_pool(name="ps_ctx", bufs=2, space="PSUM") as ps_ctx,
            tc.tile_pool(name="dramp", bufs=2, space="DRAM") as dramp,
        ):
            # ---- input loads: x on the sync queue, weights on the scalar
            # HWDGE queue so the two DMA queues drain in parallel.
            # Projections run with f32r operands (1 cyc/row at N=512) so the
            # f32 weight staging feeds the PE directly — no bf16 weight
            # casts.
            qT = persist.tile([65, 512], f32r, tag="qT")
            kT = persist.tile([65, 512], f32r, tag="kT")
            vT = persist.tile([65, 512], f32r, tag="vT")
            xins = {}
            wstgs = {}
            for name, x_d, w_d, b_d in (
                ("q", q, qw_w, qw_b),
                ("k", k, kw_w, kw_b),
                ("v", v, vw_w, vw_b),
            ):
                xin = inp.tile([128, 4, 64], f32, tag="xin", name=f"xin_{name}")
                nc.sync.dma_start(
                    out=xin, in_=x_d.rearrange("(t p) d -> p t d", p=128)
                )
                xins[name] = xin
                stg = inp.tile([65, 512], f32, tag="wstg", name=f"wstg_{name}")
                nc.scalar.dma_start(out=stg[0:64, :], in_=w_d)
                nc.scalar.dma_start(out=stg[64:65, :], in_=b_d[None, :])
                wstgs[name] = stg
            # f32r rounding passes for the weights (engine-rounded producers
            # are required for f32r matmul operands, and 32/16-bit operand
            # mixing is rejected by walrus).
            wrs = {}
            for i, name in enumerate(("q", "k", "v")):
                wr = inp.tile([65, 512], f32r, tag="wr", name=f"wr_{name}")
                if name == "k":
                    nc.vector.tensor_copy(out=wr, in_=wstgs[name])
                else:
                    nc.scalar.copy(out=wr, in_=wstgs[name])
                wrs[name] = wr

            # ---- constants (gpsimd/ACT, overlap the DMAs) ----
            ones_b = persist.tile([1, 128], bf16, tag="ones_b")
            nc.gpsimd.memset(ones_b, 1.0)
            # PE warm-up burst: ~4us of K=1 matmuls during the input-DMA
            # wait.  The HAM clock gate needs ~3.4us of sustained PE
            # activity to lift the PE from 1.2 to 2.4 GHz; without this the
            # whole setup stream (transposes + projections) runs cold.
            wps = ps.tile([128, 512], f32, tag="st", name="warmup_ps")
            for i in range(20):
                nc.tensor.matmul(
                    wps[:, 128 * (i % 4) : 128 * (i % 4) + 128],
                    lhsT=ones_b,
                    rhs=ones_b,
                    start=True,
                    stop=True,
                    skip_group_check=True,
                )
            ident = persist.tile([128, 128], f32, tag="ident")
            make_identity(nc, ident)
            ones_row = persist.tile([1, 512], f32, tag="ones_row")
            nc.gpsimd.memset(ones_row, 1.0)
            # dummy exp to pull the ACT table load into the setup phase
            warm = persist.tile([1, 16], f32, tag="warm")
            nc.scalar.activation(warm, ones_row[:, 0:16], AF.Exp, scale=1.0)

            def transposes(name, xT):
                nc.vector.tensor_copy(out=xT[64:65, :], in_=ones_row)
                for t in range(4):
                    tp = ps.tile([64, 128], f32, tag="st", name=f"tp_{name}{t}")
                    nc.tensor.transpose(tp, xins[name][:, t, :], ident)
                    nc.vector.tensor_copy(
                        out=xT[0:64, 128 * t : 128 * t + 128], in_=tp
                    )

            # ---- Q chain first: projections, chunk-0 dup ----
            # one M=128 matmul produces QpT for chunk pair (2m, 2m+1):
            # partitions 0:64 = even chunk, 64:128 = odd chunk; casts go to
            # the matching partition range of Qdup (split across DVE and the
            # idle ACT engine), missing halves filled by partition-moving
            # SBUF DMAs.
            Qdup = persist.tile([128, 4096], bf16, tag="Qdup")

            def qproj(m, eng):
                pst = ps.tile([128, 512], f32, tag="st", name=f"qp{m}")
                nc.tensor.matmul(
                    pst,
                    lhsT=wrs["q"][:, 128 * m : 128 * m + 128],
                    rhs=qT[:],
                    start=True,
                    stop=True,
                )
                ce, co = 2 * m, 2 * m + 1
                dst_e = Qdup[0:64, 512 * ce : 512 * ce + 512]
                dst_o = Qdup[64:128, 512 * co : 512 * co + 512]
                if eng == "act":
                    nc.scalar.copy(out=dst_e, in_=pst[0:64, :])
                    nc.vector.tensor_copy(out=dst_o, in_=pst[64:128, :])
                elif eng == "dve2":
                    nc.vector.tensor_copy(out=dst_e, in_=pst[0:64, :])
                    nc.vector.tensor_copy(out=dst_o, in_=pst[64:128, :])
                else:
                    nc.vector.tensor_copy(out=dst_e, in_=pst[0:64, :])
                    nc.scalar.copy(out=dst_o, in_=pst[64:128, :])

            transposes("q", qT)
            qproj(0, "dve2")
            nc.sync.dma_start(out=Qdup[64:128, 0:512], in_=Qdup[0:64, 0:512])

            # ---- K chain ----
            transposes("k", kT)
            # KpT_g [128, 512] bf16: partitions 0:64 = c=2g, 64:128 = c=2g+1
            KpT = []

            def kproj(g):
                pst = ps.tile([128, 512], f32, tag="st", name=f"kp{g}")
                nc.tensor.matmul(
                    pst,
                    lhsT=wrs["k"][:, 128 * g : 128 * g + 128],
                    rhs=kT[:],
                    start=True,
                    stop=True,
                )
                sb = persist.tile([128, 512], bf16, tag=f"KpT{g}")
                if g % 2 == 0:
                    nc.vector.tensor_copy(out=sb, in_=pst)
                else:
                    nc.scalar.copy(out=sb, in_=pst)
                KpT.append(sb)

            kproj(0)
            # chunk-0-critical projections (qproj0/kproj0) are now queued;
            # the rest follow — chunk 1+ needs them only ~15us later.
            for m in range(1, 4):
                qproj(m, "act" if m % 2 == 0 else "dve")
            Qd4 = Qdup[:].rearrange("p (m two x) -> p m two x", two=2, x=512)
            nc.sync.dma_start(
                out=Qd4[64:128, 1:4, 0, :], in_=Qd4[0:64, 1:4, 0, :]
            )
            nc.sync.dma_start(
                out=Qd4[0:64, 0:4, 1, :], in_=Qd4[64:128, 0:4, 1, :]
            )
            for _g in (1, 2, 3):
                kproj(_g)

            # ---- V chain (emitted mid-loop: scores don't need V, so the
            # first chunk's pairs start while V projects) ----
            # V with interleaved ones columns, bf16:
            # Va_u[s, 65c + j] = Vp_u[s, 64c + j] for j<64, 1.0 for j=64
            Va = []

            def vchain():
                transposes("v", vT)
                for u in range(4):
                    pst = ps.tile([128, 512], f32, tag="st", name=f"vp{u}")
                    nc.tensor.matmul(
                        pst,
                        lhsT=vT[:, 128 * u : 128 * u + 128],
                        rhs=wrs["v"][:],
                        start=True,
                        stop=True,
                    )
                    va = persist.tile([128, 520], bf16, tag=f"Va{u}")
                    nc.gpsimd.memset(va, 1.0)
                    vdst = va[:].rearrange("p (c jj) -> p c jj", c=8)[:, :, 0:64]
                    vsrc = pst[:].rearrange("p (c j) -> p c j", c=8)
                    if u % 2 == 0:
                        nc.scalar.copy(out=vdst, in_=vsrc)
                    else:
                        nc.vector.tensor_copy(out=vdst, in_=vsrc)
                    Va.append(va)

            # ---- main attention loop ----
            # units kt = 0..31 (key tiles of 128 rows); pairs (8g+u, 8g+4+u)
            # pack as tile_position rows 0/64.
            unit_order = []
            for g in range(4):
                for u in range(4):
                    unit_order.append(8 * g + u)
                    unit_order.append(8 * g + 4 + u)

            ctxN = persist.tile([64, 4096], bf16, tag="ctxN")
            ctx_tiles = {}
            av_issued = {r1c: 0 for r1c in range(8)}
            pending = []  # (r1c, e_tile, [(slot, kt), (slot, kt)])

            def emit_avs(rec_):
                r1c, e, units = rec_
                # lazy ctx allocation: the ring slot's previous occupant
                # (chunk r1c-2) must have its reader (norm_finish) emitted
                # BEFORE this allocation so the pool wires the dependency.
                if r1c not in ctx_tiles:
                    ctx_tiles[r1c] = ps_ctx.tile(
                        [65, 512], f32, tag="ctx", name=f"ctx{r1c}"
                    )
                ctx_ps = ctx_tiles[r1c]
                for slot, kt in units:
                    c, u = kt // 4, kt % 4
                    i = av_issued[r1c]
                    nc.tensor.matmul(
                        ctx_ps,
                        lhsT=Va[u][:, 65 * c : 65 * c + 65],
                        rhs=e[:, 512 * slot : 512 * slot + 512],
                        start=(i == 0),
                        stop=(i == 31),
                    )
                    av_issued[r1c] = i + 1

            r128b = normp.tile([128, 4], f32, tag="r128b")
            norm_state = {}  # r1c -> dict of live tiles between phases

            # The normalize chain is 4 DMA hops of ~2.8us completion latency
            # each; emitting it in one piece head-blocks the strict-FIFO
            # DVE/ACT queues on DMA waits (the mul waits the broadcast).  It
            # is split into three phases spread across the next two chunks
            # so every engine op finds its inputs already resident.
            def norm_start(r1c):
                ctx_ps = ctx_tiles[r1c]
                dnrow = normp.tile([1, 512], f32, tag="dnrow")
                nc.scalar.copy(out=dnrow, in_=ctx_ps[64:65, :])
                d_dram = dramp.tile([1, 512], f32, tag="d_dram")
                nc.scalar.dma_start(out=d_dram, in_=dnrow)
                if r1c == 7:
                    # tail chunk: skip the broadcast round-trip.  Load
                    # 1/denom per-PARTITION (r128b[p, t] = 1/denom[128t+p])
                    # and scale this chunk's output-projection contribution
                    # per partition instead.  ctxN keeps unnormalized ctx.
                    d128 = normp.tile([128, 4], f32, tag="d128b_in")
                    nc.scalar.dma_start(
                        out=d128,
                        in_=d_dram[0, :].rearrange("(t p) -> p t", p=128),
                    )
                    nc.vector.reciprocal(r128b[:], d128[:])
                    nc.vector.tensor_copy(
                        out=ctxN[:, 512 * 7 : 512 * 8], in_=ctx_ps[0:64, :]
                    )
                    ctx_tiles.pop(r1c)
                    return
                d128 = normp.tile([128, 4], f32, tag="d128")
                nc.sync.dma_start(
                    out=d128, in_=d_dram[0, :].rearrange("(p f) -> p f", p=128)
                )
                norm_state[r1c] = d128

            def norm_mid(r1c):
                d128 = norm_state.pop(r1c)
                r128 = normp.tile([128, 4], f32, tag="r128")
                nc.vector.reciprocal(r128[:], d128[:])
                r_dram = dramp.tile([1, 512], f32, tag="r_dram")
                nc.sync.dma_start(
                    out=r_dram[0, :].rearrange("(p f) -> p f", p=128), in_=r128
                )
                rec_bc = normp.tile([64, 512], f32, tag="recbc")
                rd = r_dram[0, :]
                nc.sync.dma_start(
                    out=rec_bc,
                    in_=bass.AP(
                        tensor=rd.tensor,
                        offset=rd.offset,
                        ap=[[0, 64]] + list(rd.ap),
                    ),
                )
                norm_state[r1c] = rec_bc

            def norm_finish(r1c):
                rec_bc = norm_state.pop(r1c)
                ctx_ps = ctx_tiles.pop(r1c)
                nc.vector.tensor_mul(
                    out=ctxN[:, 512 * r1c : 512 * r1c + 512],
                    in0=ctx_ps[0:64, :],
                    in1=rec_bc,
                )

            def drain(limit):
                while len(pending) > limit:
                    rec_ = pending.pop(0)
                    emit_avs(rec_)
                    if av_issued[rec_[0]] == 32:
                        norm_start(rec_[0])

            for r1c in range(8):
                dve_set = _dve_pairs(r1c)
                for p in range(16):
                    if p == 3 and r1c >= 2 and (r1c - 2) in norm_state:
                        # the ctx bank of chunk r1c-2 must free before this
                        # chunk's first AV (popped in this pair's drain)
                        norm_finish(r1c - 2)
                    if p == 11 and (r1c - 1) in norm_state:
                        norm_mid(r1c - 1)
                    kt_a = unit_order[2 * p]
                    kt_b = unit_order[2 * p + 1]
                    pair_tile = ps.tile([128, 1024], f32, tag="st")
                    for kt, half in ((kt_a, 0), (kt_b, 1)):
                        c, u = kt // 4, kt % 4
                        g = c // 2
                        rowpos = 64 * (c % 2)
                        nc.tensor.matmul(
                            pair_tile[:, 512 * half : 512 * half + 512],
                            lhsT=KpT[g][
                                rowpos : rowpos + 64, 128 * u : 128 * u + 128
                            ],
                            rhs=Qdup[
                                rowpos : rowpos + 64,
                                512 * r1c : 512 * r1c + 512,
                            ],
                            start=True,
                            stop=True,
                            tile_position=(rowpos, 0),
                        )
                    e = epool.tile([128, 1024], bf16, tag="e")
                    if p in dve_set:
                        nc.vector.tensor_scalar(
                            out=e[:].bitcast(i16),
                            in0=pair_tile[:],
                            scalar1=SCH_A,
                            scalar2=SCH_B,
                            op0=ALU.mult,
                            op1=ALU.add,
                        )
                    else:
                        nc.scalar.activation(
                            e[:], pair_tile[:], AF.Exp, scale=0.125
                        )
                    pending.append((r1c, e, [(0, kt_a), (1, kt_b)]))
                    if r1c == 0 and p == 1:
                        # V projections slot in behind the first two pairs:
                        # their consumers run while the PE projects V, and
                        # the first AVs (pair 0) only fire at p==3.
                        vchain()
                    # drain two records at a time on every other pair: AV
                    # bursts of 4 matmuls halve the exposed LDWEIGHTS
                    # serialization at score<->AV transitions on the PE.
                    if p % 2 == 1:
                        drain(AV_DELAY_P)
            drain(0)
            norm_finish(6)

            # ---- output projection (bf16) ----
            # ow_sb[d', 64c+j] = ow_w[64c+d', j], bf16
            ow_stg = persist.tile([64, 8, 64], f32, tag="ow_stg")
            nc.sync.dma_start(
                out=ow_stg, in_=ow_w.rearrange("(c d) j -> d c j", d=64)
            )
            ow_sb = persist.tile([64, 512], bf16, tag="ow_sb")
            nc.vector.tensor_copy(
                out=ow_sb, in_=ow_stg.rearrange("d c j -> d (c j)")
            )
            # bias replicated 4x so ONE start=True matmul covers all four
            # 64-col row-tile slices of the accumulator bank (start clears
            # the bank's has_written state, so one start per bank only).
            owb_stg = persist.tile([1, 4, 64], f32, tag="owb_stg")
            ob_src = ow_b[None, :]
            nc.sync.dma_start(
                out=owb_stg,
                in_=bass.AP(
                    tensor=ob_src.tensor,
                    offset=ob_src.offset,
                    ap=[list(ob_src.ap[0]), [0, 4]] + list(ob_src.ap[1:]),
                ),
            )
            owb_sb = persist.tile([1, 256], bf16, tag="owb_sb")
            nc.vector.tensor_copy(
                out=owb_sb, in_=owb_stg.rearrange("p t d -> p (t d)")
            )

            # one [128, 256] PSUM accumulator for bias + chunks 0..6; chunk
            # 7 projects separately (unnormalized) into ps7 and is scaled
            # per-partition by r128b, so only 2 DMA hops gate the tail.
            out_ps = ps.tile([128, 1024], f32, tag="st", name="out_ps")
            nc.tensor.matmul(
                out_ps[:, 0:256],
                lhsT=ones_b,
                rhs=owb_sb,
                start=True,
                stop=False,
                skip_group_check=True,
            )
            for c in range(7):
                for t in range(4):
                    nc.tensor.matmul(
                        out_ps[:, 64 * t : 64 * t + 64],
                        lhsT=ctxN[:, 512 * c + 128 * t : 512 * c + 128 * t + 128],
                        rhs=ow_sb[:, 64 * c : 64 * c + 64],
                        start=False,
                        stop=(c == 6),
                        skip_group_check=True,
                    )
            ps7 = ps.tile([128, 1024], f32, tag="st", name="ps7")
            for t in range(4):
                nc.tensor.matmul(
                    ps7[:, 64 * t : 64 * t + 64],
                    lhsT=ctxN[:, 512 * 7 + 128 * t : 512 * 7 + 128 * t + 128],
                    rhs=ow_sb[:, 64 * 7 : 64 * 7 + 64],
                    start=(t == 0),
                    stop=(t == 3),
                    skip_group_check=True,
                )
            ob7 = outp.tile([128, 256], f32, tag="ob7")
            for t in range(4):
                nc.vector.tensor_scalar(
                    out=ob7[:, 64 * t : 64 * t + 64],
                    in0=ps7[:, 64 * t : 64 * t + 64],
                    scalar1=r128b[:, t : t + 1],
                    scalar2=None,
                    op0=ALU.mult,
                )
            out_f = outp.tile([128, 256], f32, tag="out_f")
            nc.vector.tensor_add(out=out_f, in0=out_ps[:, 0:256], in1=ob7[:])
            nc.sync.dma_start(
                out=out.rearrange("(t p) d -> p t d", p=128),
                in_=out_f[:].rearrange("p (t d) -> p t d", t=4),
            )

    nc.compile()
    return nc


def _get_built():
    global _BUILT
    if _BUILT is None:
        _BUILT = _build()
    return _BUILT


def _make_in_maps(inputs):
    f32 = np.float32
    full = {k: np.ascontiguousarray(np.asarray(v, dtype=f32)) for k, v in inputs.items()}
    in_maps = []
    for i in range(N_CORES):
        sl = slice(B * i, B * (i + 1))
        in_maps.append(
            {
                "q": full["q"][sl],
                "k": full["k"][sl],
                "v": full["v"][sl],
                "qw_w": full["qw_w"],
                "qw_b": full["qw_b"],
                "kw_w": full["kw_w"],
                "kw_b": full["kw_b"],
                "vw_w": full["vw_w"],
                "vw_b": full["vw_b"],
                "ow_w": full["ow_w"],
                "ow_b": full["ow_b"],
            }
        )
    return in_maps


def kernel(**inputs):
    from concourse.bass_utils import run_bass_kernel_spmd

    nc = _get_built()
    res = run_bass_kernel_spmd(nc, _make_in_maps(inputs), list(range(N_CORES)))
    return np.concatenate([res.results[i]["out"] for i in range(N_CORES)], axis=0)


# revision 31
# speedup vs baseline: 1.0090x; 1.0090x over previous
"""Trainium2 Bass kernel for nn_MultiHeadAttention_53266184405720.

Key structural fact: the reference does a raw ``.reshape(h, -1, d)`` on the
[4096, 512] projection output, so "head" h consumes exactly projection rows
[512h, 512h+512) — i.e. sequence rows [512h, 512h+512).  The whole module is
block-diagonal over 512-row sequence blocks: core h computes output rows
[512h, 512h+512) from input rows [512h, 512h+512) plus the (replicated)
weights.  No cross-core communication is needed.

Within a block, with the permutation r~ = c*512 + s (c = column-block of the
projection, s = row), head-reshaped Q/K/V become column-block stacks of the
projection, softmax is permutation-invariant over keys, and the context
unpermutes back into the output projection's contraction.  The transposed
projection layout [64, 512] per column-block c therefore yields every
attention operand as a zero-cost sub-AP.

Perf design (HW-trace driven; baseline 163us -> ~140us):
 - exp offload: ACT (1 elem/cyc/lane) alone needs ~135us for the 16.8M
   score exps and paced the v1 kernel.  Exp is split between ACT (exact
   spline exp) and DVE (one fused tensor_scalar: i16 = round(A*s + B),
   bit-cast to bf16 — Schraudolph in the bf16 exponent/mantissa fields,
   rms err ~1.8% on the offloaded ~47% of keys; HW rounds to nearest).
   Scores issue as packed pairs (tile_position rows 0/64), one pair per
   [128,1024] PSUM tile (3-deep ring, 6 banks), consumers alternating
   ACT/DVE (9:7 / 8:8) so the PE (bf16 scores + AV) becomes the pacer.
 - PE stream: AV matmuls drain in bursts of 4 every other pair — the
   score<->AV transitions expose LDWEIGHTS serialization (~200ns each),
   bursts halve them.  Chunk-0-critical projections (qproj0/kproj0) are
   emitted before the remaining projections; the V chain rides inside
   chunk 0 (scores don't need V until the first AV burst).
 - softmax 1/denom rides the AV matmul as a ones-column (ctx row 64).
   The [1,512] row reciprocal would cost 3.4us/chunk on DVE at 8 cyc/elem;
   instead the row bounces through DRAM as [128,4] (150ns recip) and back
   for the partition-broadcast load (partition-step-0 APs are only legal
   from DRAM here).  Each DMA hop has ~2.8us completion latency, so the
   chain is emitted in three phases across the next two chunks (start:
   copy+2 hops; mid: recip+2 hops; finish: the ctxN multiply) — emitting
   it in one piece head-blocks the strict-FIFO DVE queue and stalls the
   PE ~1-2us per chunk.
 - tail: the output projection accumulates bias (one start=True matmul —
   a later start on the same bank wipes previous slices' has_written) and
   chunks 0..6 into one [128,256] PSUM tile during chunk 7; chunk 7
   projects unnormalized and is scaled per-partition (tensor_scalar with
   an AP scalar, r128b[p,t] = 1/denom[128t+p]) so only 2 DMA hops gate
   the tail instead of 4.
"""

import numpy as np

SEQ = 4096
D = 64
HEADS = 8
B = SEQ // HEADS  # 512 rows per core
N_CORES = 8

# Schraudolph exp for bf16 bit pattern: i16 = round(s * SCH_A + SCH_B),
# bitcast i16 -> bf16 approximates exp(s/8).  SCH_A = (2^7/ln2)/8;
# SCH_B = 127*2^7 - C with C tuned to minimize rms relative error (~1.78%).
SCH_A = 23.083128655
SCH_B = 16256.0 - 7.5

AV_DELAY_P = 2  # pending pairs kept before AV drain (drained 2 at a time)


def _dve_pairs(r1c):
    # even chunks: 7 DVE pairs at odd slots (<=13); odd chunks: 8 at even
    # slots.  Chosen so chunk boundaries keep the two consumers interleaved.
    if r1c % 2 == 0:
        return {1, 3, 5, 7, 9, 11, 13}
    return {0, 2, 4, 6, 8, 10, 12, 14}


_BUILT = None


def _build():
    import concourse.bass as bass
    import concourse.tile as tile
    from concourse import bacc, mybir
    from concourse.masks import make_identity

    f32 = mybir.dt.float32
    f32r = mybir.dt.float32r
    bf16 = mybir.dt.bfloat16
    i16 = mybir.dt.int16
    AF = mybir.ActivationFunctionType
    ALU = mybir.AluOpType

    nc = bacc.Bacc(
        "TRN2",
        target_bir_lowering=False,
        debug=False,
        enable_asserts=True,
        num_devices=N_CORES,
    )

    q = nc.dram_tensor("q", [B, D], f32, kind="ExternalInput").ap()
    k = nc.dram_tensor("k", [B, D], f32, kind="ExternalInput").ap()
    v = nc.dram_tensor("v", [B, D], f32, kind="ExternalInput").ap()
    qw_w = nc.dram_tensor("qw_w", [D, 512], f32, kind="ExternalInput").ap()
    qw_b = nc.dram_tensor("qw_b", [512], f32, kind="ExternalInput").ap()
    kw_w = nc.dram_tensor("kw_w", [D, 512], f32, kind="ExternalInput").ap()
    kw_b = nc.dram_tensor("kw_b", [512], f32, kind="ExternalInput").ap()
    vw_w = nc.dram_tensor("vw_w", [D, 512], f32, kind="ExternalInput").ap()
    vw_b = nc.dram_tensor("vw_b", [512], f32, kind="ExternalInput").ap()
    ow_w = nc.dram_tensor("ow_w", [512, D], f32, kind="ExternalInput").ap()
    ow_b = nc.dram_tensor("ow_b", [D], f32, kind="ExternalInput").ap()
    out = nc.dram_tensor("out", [B, D], f32, kind="ExternalOutput").ap()

    with tile.TileContext(nc) as tc:
        with (
            tc.tile_pool(name="persist", bufs=1) as persist,
            tc.tile_pool(name="inp", bufs=3) as inp,
            tc.tile_pool(name="epool", bufs=6) as epool,
            tc.tile_pool(name="norm", bufs=2) as normp,
            tc.tile_pool(name="outp", bufs=1) as outp,
            tc.tile_pool(name="ps", bufs=3, space="PSUM") as ps,
            tc.tile_pool(name="ps_ctx", bufs=2, space="PSUM") as ps_ctx,
            tc.tile_pool(name="dramp", bufs=2, space="DRAM") as dramp,
        ):
            # ---- input loads: x on the sync queue, weights on the scalar
            # HWDGE queue so the two DMA queues drain in parallel.
            qT = persist.tile([65, 512], bf16, tag="qT")
            kT = persist.tile([65, 512], bf16, tag="kT")
            vT = persist.tile([65, 512], bf16, tag="vT")
            xins = {}
            wstgs = {}
            for name, x_d, w_d, b_d in (
                ("q", q, qw_w, qw_b),
                ("k", k, kw_w, kw_b),
                ("v", v, vw_w, vw_b),
            ):
                xin = inp.tile([128, 4, 64], f32, tag="xin", name=f"xin_{name}")
                nc.sync.dma_start(
                    out=xin, in_=x_d.rearrange("(t p) d -> p t d", p=128)
                )
                xins[name] = xin
                stg = inp.tile([65, 512], f32, tag="wstg", name=f"wstg_{name}")
                nc.scalar.dma_start(out=stg[0:64, :], in_=w_d)
                nc.scalar.dma_start(out=stg[64:65, :], in_=b_d[None, :])
                wstgs[name] = stg
            # bf16 weight casts: 1-pass ~107ns LDWEIGHTS vs ~300ns 2-pass
            # for 4-byte weights; projections are all-bf16 (walrus rejects
            # 32/16-bit operand mixing).
            wrs = {}
            for i, name in enumerate(("q", "k", "v")):
                wr = inp.tile([65, 512], bf16, tag="wr", name=f"wr_{name}")
                if name == "k":
                    nc.vector.tensor_copy(out=wr, in_=wstgs[name])
                else:
                    nc.scalar.copy(out=wr, in_=wstgs[name])
                wrs[name] = wr

            # ---- constants (gpsimd/ACT, overlap the DMAs) ----
            ones_b = persist.tile([1, 128], bf16, tag="ones_b")
            nc.gpsimd.memset(ones_b, 1.0)
            # PE warm-up burst: ~4us of K=1 matmuls during the input-DMA
            # wait.  The HAM clock gate needs ~3.4us of sustained PE
            # activity to lift the PE from 1.2 to 2.4 GHz; without this the
            # whole setup stream (transposes + projections) runs cold.
            wps = ps.tile([128, 512], f32, tag="st", name="warmup_ps")
            for i in range(20):
                nc.tensor.matmul(
                    wps[:, 128 * (i % 4) : 128 * (i % 4) + 128],
                    lhsT=ones_b,
                    rhs=ones_b,
                    start=True,
                    stop=True,
                    skip_group_check=True,
                )
            ident = persist.tile([128, 128], f32, tag="ident")
            make_identity(nc, ident)
            ones_row = persist.tile([1, 512], f32, tag="ones_row")
            nc.gpsimd.memset(ones_row, 1.0)
            # dummy exp to pull the ACT table load into the setup phase
            warm = persist.tile([1, 16], f32, tag="warm")
            nc.scalar.activation(warm, ones_row[:, 0:16], AF.Exp, scale=1.0)

            def transposes(name, xT):
                nc.vector.tensor_copy(out=xT[64:65, :], in_=ones_row)
                for t in range(4):
                    tp = ps.tile([64, 128], f32, tag="st", name=f"tp_{name}{t}")
                    nc.tensor.transpose(tp, xins[name][:, t, :], ident)
                    nc.vector.tensor_copy(
                        out=xT[0:64, 128 * t : 128 * t + 128], in_=tp
                    )

            # ---- Q chain first: projections, chunk-0 dup ----
            # one M=128 matmul produces QpT for chunk pair (2m, 2m+1):
            # partitions 0:64 = even chunk, 64:128 = odd chunk; casts go to
            # the matching partition range of Qdup (split across DVE and the
            # idle ACT engine), missing halves filled by partition-moving
            # SBUF DMAs.
            Qdup = persist.tile([128, 4096], bf16, tag="Qdup")

            def qproj(m, eng):
                pst = ps.tile([128, 512], f32, tag="st", name=f"qp{m}")
                nc.tensor.matmul(
                    pst,
                    lhsT=wrs["q"][:, 128 * m : 128 * m + 128],
                    rhs=qT[:],
                    start=True,
                    stop=True,
                )
                ce, co = 2 * m, 2 * m + 1
                dst_e = Qdup[0:64, 512 * ce : 512 * ce + 512]
                dst_o = Qdup[64:128, 512 * co : 512 * co + 512]
                if eng == "act":
                    nc.scalar.copy(out=dst_e, in_=pst[0:64, :])
                    nc.vector.tensor_copy(out=dst_o, in_=pst[64:128, :])
                elif eng == "dve2":
                    nc.vector.tensor_copy(out=dst_e, in_=pst[0:64, :])
                    nc.vector.tensor_copy(out=dst_o, in_=pst[64:128, :])
                else:
                    nc.vector.tensor_copy(out=dst_e, in_=pst[0:64, :])
                    nc.scalar.copy(out=dst_o, in_=pst[64:128, :])

            transposes("q", qT)
            qproj(0, "dve2")
            nc.sync.dma_start(out=Qdup[64:128, 0:512], in_=Qdup[0:64, 0:512])

            # ---- K chain ----
            transposes("k", kT)
            # KpT_g [128, 512] bf16: partitions 0:64 = c=2g, 64:128 = c=2g+1
            KpT = []

            def kproj(g):
                pst = ps.tile([128, 512], f32, tag="st", name=f"kp{g}")
                nc.tensor.matmul(
                    pst,
                    lhsT=wrs["k"][:, 128 * g : 128 * g + 128],
                    rhs=kT[:],
                    start=True,
                    stop=True,
                )
                sb = persist.tile([128, 512], bf16, tag=f"KpT{g}")
                if g % 2 == 0:
                    nc.vector.tensor_copy(out=sb, in_=pst)
                else:
                    nc.scalar.copy(out=sb, in_=pst)
                KpT.append(sb)

            kproj(0)
            # chunk-0-critical projections (qproj0/kproj0) are now queued;
            # the rest follow — chunk 1+ needs them only ~15us later.
            for m in range(1, 4):
                qproj(m, "act" if m % 2 == 0 else "dve")
            Qd4 = Qdup[:].rearrange("p (m two x) -> p m two x", two=2, x=512)
            nc.sync.dma_start(
                out=Qd4[64:128, 1:4, 0, :], in_=Qd4[0:64, 1:4, 0, :]
            )
            nc.sync.dma_start(
                out=Qd4[0:64, 0:4, 1, :], in_=Qd4[64:128, 0:4, 1, :]
            )
            for _g in (1, 2, 3):
                kproj(_g)

            # ---- V chain (emitted mid-loop: scores don't need V, so the
            # first chunk's pairs start while V projects) ----
            # V with interleaved ones columns, bf16:
            # Va_u[s, 65c + j] = Vp_u[s, 64c + j] for j<64, 1.0 for j=64
            Va = []

            def vchain():
                transposes("v", vT)
                for u in range(4):
                    pst = ps.tile([128, 512], f32, tag="st", name=f"vp{u}")
                    nc.tensor.matmul(
                        pst,
                        lhsT=vT[:, 128 * u : 128 * u + 128],
                        rhs=wrs["v"][:],
                        start=True,
                        stop=True,
                    )
                    va = persist.tile([128, 520], bf16, tag=f"Va{u}")
                    nc.gpsimd.memset(va, 1.0)
                    vdst = va[:].rearrange("p (c jj) -> p c jj", c=8)[:, :, 0:64]
                    vsrc = pst[:].rearrange("p (c j) -> p c j", c=8)
                    if u % 2 == 0:
                        nc.scalar.copy(out=vdst, in_=vsrc)
                    else:
                        nc.vector.tensor_copy(out=vdst, in_=vsrc)
                    Va.append(va)

            # ---- main attention loop ----
            # units kt = 0..31 (key tiles of 128 rows); pairs (8g+u, 8g+4+u)
            # pack as tile_position rows 0/64.
            unit_order = []
            for g in range(4):
                for u in range(4):
                    unit_order.append(8 * g + u)
                    unit_order.append(8 * g + 4 + u)

            ctxN = persist.tile([64, 4096], bf16, tag="ctxN")
            ctx_tiles = {}
            av_issued = {r1c: 0 for r1c in range(8)}
            pending = []  # (r1c, e_tile, [(slot, kt), (slot, kt)])

            def emit_avs(rec_):
                r1c, e, units = rec_
                # lazy ctx allocation: the ring slot's previous occupant
                # (chunk r1c-2) must have its reader (norm_finish) emitted
                # BEFORE this allocation so the pool wires the dependency.
                if r1c not in ctx_tiles:
                    ctx_tiles[r1c] = ps_ctx.tile(
                        [65, 512], f32, tag="ctx", name=f"ctx{r1c}"
                    )
                ctx_ps = ctx_tiles[r1c]
                for slot, kt in units:
                    c, u = kt // 4, kt % 4
                    i = av_issued[r1c]
                    nc.tensor.matmul(
                        ctx_ps,
                        lhsT=Va[u][:, 65 * c : 65 * c + 65],
                        rhs=e[:, 512 * slot : 512 * slot + 512],
                        start=(i == 0),
                        stop=(i == 31),
                    )
                    av_issued[r1c] = i + 1

            r128b = normp.tile([128, 4], f32, tag="r128b")
            norm_state = {}  # r1c -> dict of live tiles between phases

            # The normalize chain is 4 DMA hops of ~2.8us completion latency
            # each; emitting it in one piece head-blocks the strict-FIFO
            # DVE/ACT queues on DMA waits (the mul waits the broadcast).  It
            # is split into three phases spread across the next two chunks
            # so every engine op finds its inputs already resident.
            def norm_start(r1c):
                ctx_ps = ctx_tiles[r1c]
                dnrow = normp.tile([1, 512], f32, tag="dnrow")
                nc.scalar.copy(out=dnrow, in_=ctx_ps[64:65, :])
                d_dram = dramp.tile([1, 512], f32, tag="d_dram")
                nc.scalar.dma_start(out=d_dram, in_=dnrow)
                if r1c == 7:
                    # tail chunk: skip the broadcast round-trip.  Load
                    # 1/denom per-PARTITION (r128b[p, t] = 1/denom[128t+p])
                    # and scale this chunk's output-projection contribution
                    # per partition instead.  ctxN keeps unnormalized ctx.
                    d128 = normp.tile([128, 4], f32, tag="d128b_in")
                    nc.scalar.dma_start(
                        out=d128,
                        in_=d_dram[0, :].rearrange("(t p) -> p t", p=128),
                    )
                    nc.vector.reciprocal(r128b[:], d128[:])
                    nc.vector.tensor_copy(
                        out=ctxN[:, 512 * 7 : 512 * 8], in_=ctx_ps[0:64, :]
                    )
                    ctx_tiles.pop(r1c)
                    return
                d128 = normp.tile([128, 4], f32, tag="d128")
                nc.sync.dma_start(
                    out=d128, in_=d_dram[0, :].rearrange("(p f) -> p f", p=128)
                )
                norm_state[r1c] = d128

            def norm_mid(r1c):
                d128 = norm_state.pop(r1c)
                r128 = normp.tile([128, 4], f32, tag="r128")
                nc.vector.reciprocal(r128[:], d128[:])
                r_dram = dramp.tile([1, 512], f32, tag="r_dram")
                nc.sync.dma_start(
                    out=r_dram[0, :].rearrange("(p f) -> p f", p=128), in_=r128
                )
                rec_bc = normp.tile([64, 512], f32, tag="recbc")
                rd = r_dram[0, :]
                nc.sync.dma_start(
                    out=rec_bc,
                    in_=bass.AP(
                        tensor=rd.tensor,
                        offset=rd.offset,
                        ap=[[0, 64]] + list(rd.ap),
                    ),
                )
                norm_state[r1c] = rec_bc

            def norm_finish(r1c):
                rec_bc = norm_state.pop(r1c)
                ctx_ps = ctx_tiles.pop(r1c)
                nc.vector.tensor_mul(
                    out=ctxN[:, 512 * r1c : 512 * r1c + 512],
                    in0=ctx_ps[0:64, :],
                    in1=rec_bc,
                )

            def drain(limit):
                while len(pending) > limit:
                    rec_ = pending.pop(0)
                    emit_avs(rec_)
                    if av_issued[rec_[0]] == 32:
                        norm_start(rec_[0])

            for r1c in range(8):
                dve_set = _dve_pairs(r1c)
                for p in range(16):
                    if p == 3 and r1c >= 2 and (r1c - 2) in norm_state:
                        # the ctx bank of chunk r1c-2 must free before this
                        # chunk's first AV (popped in this pair's drain)
                        norm_finish(r1c - 2)
                    if p == 11 and (r1c - 1) in norm_state:
                        norm_mid(r1c - 1)
                    kt_a = unit_order[2 * p]
                    kt_b = unit_order[2 * p + 1]
                    pair_tile = ps.tile([128, 1024], f32, tag="st")
                    for kt, half in ((kt_a, 0), (kt_b, 1)):
                        c, u = kt // 4, kt % 4
                        g = c // 2
                        rowpos = 64 * (c % 2)
                        nc.tensor.matmul(
                            pair_tile[:, 512 * half : 512 * half + 512],
                            lhsT=KpT[g][
                                rowpos : rowpos + 64, 128 * u : 128 * u + 128
                            ],
                            rhs=Qdup[
                                rowpos : rowpos + 64,
                                512 * r1c : 512 * r1c + 512,
                            ],
                            start=True,
                            stop=True,
                            tile_position=(rowpos, 0),
                        )
                    e = epool.tile([128, 1024], bf16, tag="e")
                    if p in dve_set:
                        nc.vector.tensor_scalar(
                            out=e[:].bitcast(i16),
                            in0=pair_tile[:],
                            scalar1=SCH_A,
                            scalar2=SCH_B,
                            op0=ALU.mult,
                            op1=ALU.add,
                        )
                    else:
                        nc.scalar.activation(
                            e[:], pair_tile[:], AF.Exp, scale=0.125
                        )
                    pending.append((r1c, e, [(0, kt_a), (1, kt_b)]))
                    if r1c == 0 and p == 1:
                        # V projections slot in behind the first two pairs:
                        # their consumers run while the PE projects V, and
                        # the first AVs (pair 0) only fire at p==3.
                        vchain()
                    # drain two records at a time on every other pair: AV
                    # bursts of 4 matmuls halve the exposed LDWEIGHTS
                    # serialization at score<->AV transitions on the PE.
                    if p % 2 == 1:
                        drain(AV_DELAY_P)
            drain(0)
            norm_finish(6)

            # ---- output projection (bf16) ----
            # ow_sb[d', 64c+j] = ow_w[64c+d', j], bf16
            ow_stg = persist.tile([64, 8, 64], f32, tag="ow_stg")
            nc.sync.dma_start(
                out=ow_stg, in_=ow_w.rearrange("(c d) j -> d c j", d=64)
            )
            ow_sb = persist.tile([64, 512], bf16, tag="ow_sb")
            nc.vector.tensor_copy(
                out=ow_sb, in_=ow_stg.rearrange("d c j -> d (c j)")
            )
            # bias replicated 4x so ONE start=True matmul covers all four
            # 64-col row-tile slices of the accumulator bank (start clears
            # the bank's has_written state, so one start per bank only).
            owb_stg = persist.tile([1, 4, 64], f32, tag="owb_stg")
            ob_src = ow_b[None, :]
            nc.sync.dma_start(
                out=owb_stg,
                in_=bass.AP(
                    tensor=ob_src.tensor,
                    offset=ob_src.offset,
                    ap=[list(ob_src.ap[0]), [0, 4]] + list(ob_src.ap[1:]),
                ),
            )
            owb_sb = persist.tile([1, 256], bf16, tag="owb_sb")
            nc.vector.tensor_copy(
                out=owb_sb, in_=owb_stg.rearrange("p t d -> p (t d)")
            )

            # one [128, 256] PSUM accumulator for bias + chunks 0..6; chunk
            # 7 projects separately (unnormalized) into ps7 and is scaled
            # per-partition by r128b, so only 2 DMA hops gate the tail.
            out_ps = ps.tile([128, 1024], f32, tag="st", name="out_ps")
            nc.tensor.matmul(
                out_ps[:, 0:256],
                lhsT=ones_b,
                rhs=owb_sb,
                start=True,
                stop=False,
                skip_group_check=True,
            )
            for c in range(7):
                for t in range(4):
                    nc.tensor.matmul(
                        out_ps[:, 64 * t : 64 * t + 64],
                        lhsT=ctxN[:, 512 * c + 128 * t : 512 * c + 128 * t + 128],
                        rhs=ow_sb[:, 64 * c : 64 * c + 64],
                        start=False,
                        stop=(c == 6),
                        skip_group_check=True,
                    )
            ps7 = ps.tile([128, 1024], f32, tag="st", name="ps7")
            for t in range(4):
                nc.tensor.matmul(
                    ps7[:, 64 * t : 64 * t + 64],
                    lhsT=ctxN[:, 512 * 7 + 128 * t : 512 * 7 + 128 * t + 128],
                    rhs=ow_sb[:, 64 * 7 : 64 * 7 + 64],
                    start=(t == 0),
                    stop=(t == 3),
                    skip_group_check=True,
                )
            ob7 = outp.tile([128, 256], f32, tag="ob7")
            for t in range(4):
                nc.vector.tensor_scalar(
                    out=ob7[:, 64 * t : 64 * t + 64],
                    in0=ps7[:, 64 * t : 64 * t + 64],
                    scalar1=r128b[:, t : t + 1],
                    scalar2=None,
                    op0=ALU.mult,
                )
            out_f = outp.tile([128, 256], f32, tag="out_f")
            nc.vector.tensor_add(out=out_f, in0=out_ps[:, 0:256], in1=ob7[:])
            nc.sync.dma_start(
                out=out.rearrange("(t p) d -> p t d", p=128),
                in_=out_f[:].rearrange("p (t d) -> p t d", t=4),
            )

    nc.compile()
    return nc


def _get_built():
    global _BUILT
    if _BUILT is None:
        _BUILT = _build()
    return _BUILT


def _make_in_maps(inputs):
    f32 = np.float32
    full = {k: np.ascontiguousarray(np.asarray(v, dtype=f32)) for k, v in inputs.items()}
    in_maps = []
    for i in range(N_CORES):
        sl = slice(B * i, B * (i + 1))
        in_maps.append(
            {
                "q": full["q"][sl],
                "k": full["k"][sl],
                "v": full["v"][sl],
                "qw_w": full["qw_w"],
                "qw_b": full["qw_b"],
                "kw_w": full["kw_w"],
                "kw_b": full["kw_b"],
                "vw_w": full["vw_w"],
                "vw_b": full["vw_b"],
                "ow_w": full["ow_w"],
                "ow_b": full["ow_b"],
            }
        )
    return in_maps


def kernel(**inputs):
    from concourse.bass_utils import run_bass_kernel_spmd

    nc = _get_built()
    res = run_bass_kernel_spmd(nc, _make_in_maps(inputs), list(range(N_CORES)))
    return np.concatenate([res.results[i]["out"] for i in range(N_CORES)], axis=0)
